# revision 1
# baseline (speedup 1.0000x reference)
"""Trainium2 Bass kernel for nn_Block_33105607917680 (gnn_message_passing).

Sharding: batch (2) x N-shard (4) over 8 cores; each core owns 2048 points
of one batch. Per LFP layer, cores compute their h-shard (x @ W, row-major)
and AllGather it into a per-batch-group [8192, 256] bf16 HBM table; KNN
neighbor features are fetched with dma_gather. Gaussian kernel weights are
computed on-device from a host-precomputed rank-7 geometric basis
(pn, pn^2, 1) via TensorE + Exp on ScalarE. The weighted k-reduction is a
block-0/1 selection matmul accumulated in PSUM. BatchNorm batch statistics
are AllReduced (sum/sumsq) across all 8 cores.

Channels are relabeled host-side (c=4g+c4 -> 64*c4+g) so the per-group
gaussian weight broadcast becomes a stride-1 read (DVE 2x mode); all weight
matrices are permuted to match and the output is unpermuted on the host.
"""
import sys
sys.path.insert(0, '/opt/trn_rl_repo')

import numpy as np
import ml_dtypes

BF = ml_dtypes.bfloat16
B, N, K, DIM, DEPTH, HID = 2, 8192, 16, 256, 4, 1024
D4 = DIM // 4
EPS = 1e-5
NCORES, SHARD = 8, 2048
NT = SHARD // 128            # point tiles per core
ROWS_T = 128 * K             # gathered rows per tile
NSLOT = ROWS_T // 128        # row slots per tile
NCH = 4                      # MLP n-chunks
CHN = SHARD // NCH           # 512

PERM = np.zeros(DIM, np.int64)
for _g in range(D4):
    for _c4 in range(4):
        PERM[64 * _c4 + _g] = 4 * _g + _c4
PERM_INV = np.argsort(PERM)


# ---------------------------------------------------------------- host prep
def _pack_inputs(inp):
    x = np.asarray(inp["x"], np.float32)
    xyz = np.asarray(inp["xyz"], np.float32)
    knn = np.asarray(inp["knn"])
    assert knn.dtype == np.int32

    rhs7 = np.zeros((128, DEPTH * 64), np.float32)
    for l in range(DEPTH):
        u = np.asarray(inp["lfp_scale"], np.float32)[l] ** 2
        c = np.asarray(inp["lfp_coor"], np.float32)[l]
        r7 = np.zeros((7, D4), np.float32)
        r7[0:3] = 2.0 * u * c.T
        r7[3:6] = -u
        r7[6] = -u * (c ** 2).sum(-1)
        for rg in range(4):
            rhs7[32 * rg:32 * rg + 7, l * 64:(l + 1) * 64] = r7

    ssb = np.zeros((128, NSLOT * 128), np.float32)
    for s in range(NSLOT):
        for p in range(128):
            ssb[p, s * 128 + s * 8 + p // 16] = 1.0 / K

    wproj = np.zeros((128, DEPTH * 2 * DIM), np.float32)
    for l in range(DEPTH):
        w = np.asarray(inp["lfp_proj"], np.float32)[l][PERM][:, PERM]
        for kt in range(2):
            wproj[:, (l * 2 + kt) * DIM:(l * 2 + kt + 1) * DIM] = w[kt * 128:(kt + 1) * 128]

    w1 = np.zeros((128, 3 * 2 * HID), np.float32)
    w2 = np.zeros((128, 3 * 8 * DIM), np.float32)
    b1 = np.zeros((128, 3 * 8), np.float32)
    mg = np.zeros((128, 3 * 2), np.float32)
    mb = np.zeros((128, 3 * 2), np.float32)
    lg = np.zeros((128, DEPTH * 2), np.float32)
    lb = np.zeros((128, DEPTH * 2), np.float32)
    for j in range(3):
        a = np.asarray(inp["mlp_w1"], np.float32)[j][PERM]
        for kt in range(2):
            w1[:, (j * 2 + kt) * HID:(j * 2 + kt + 1) * HID] = a[kt * 128:(kt + 1) * 128]
        a = np.asarray(inp["mlp_w2"], np.float32)[j][:, PERM]
        for ht in range(8):
            w2[:, (j * 8 + ht) * DIM:(j * 8 + ht + 1) * DIM] = a[ht * 128:(ht + 1) * 128]
        for ht in range(8):
            b1[:, j * 8 + ht] = np.asarray(inp["mlp_b1"], np.float32)[j][ht * 128:(ht + 1) * 128]
        gj = np.asarray(inp["mlp_gamma"], np.float32)[j][PERM]
        bj = np.asarray(inp["mlp_beta"], np.float32)[j][PERM]
        for ct in range(2):
            mg[:, j * 2 + ct] = gj[ct * 128:(ct + 1) * 128]
            mb[:, j * 2 + ct] = bj[ct * 128:(ct + 1) * 128]
    for l in range(DEPTH):
        gl = np.asarray(inp["lfp_gamma"], np.float32)[l][PERM]
        bl = np.asarray(inp["lfp_beta"], np.float32)[l][PERM]
        for ct in range(2):
            lg[:, l * 2 + ct] = gl[ct * 128:(ct + 1) * 128]
            lb[:, l * 2 + ct] = bl[ct * 128:(ct + 1) * 128]

    shared = {
        "rhs7": rhs7.astype(BF), "ssb": ssb.astype(BF), "wproj": wproj.astype(BF),
        "w1": w1.astype(BF), "w2": w2.astype(BF), "b1": b1,
        "mg": mg, "mb": mb, "lg": lg, "lb": lb,
    }

    in_maps = []
    for core in range(NCORES):
        b, sh = core // 4, core % 4
        rows = slice(sh * SHARD, (sh + 1) * SHARD)
        xT0 = np.ascontiguousarray(x[b, rows][:, PERM].T)

        nn = knn[b, rows].reshape(-1).astype(np.int64)          # [32768]
        # wrapped idx layout: per tile t, col t*128+q, partition 16g+p16
        flat = nn.astype(np.int16).reshape(NT, 128, K)          # [t, nl, k]
        flat = flat.reshape(NT, ROWS_T)                         # f = nl*16+k
        idxw = np.zeros((128, NT * 128), np.int16)
        for t in range(NT):
            w = flat[t].reshape(128, 16).T                      # [p16, q]
            for g in range(8):
                idxw[g * 16:(g + 1) * 16, t * 128:(t + 1) * 128] = w

        ctr = np.repeat(np.arange(sh * SHARD, (sh + 1) * SHARD), K)
        pn = (xyz[b, nn] - xyz[b, ctr]).T                       # [3, 32768]
        bas7 = np.concatenate([pn, pn ** 2, np.ones((1, pn.shape[1]), np.float32)], 0)
        basis = np.zeros((128, 8192), np.float32)
        for sg in range(NT * NSLOT):
            rg, cb = sg % 4, sg // 4
            basis[32 * rg:32 * rg + 7, cb * 128:(cb + 1) * 128] = \
                bas7[:, sg * 128:(sg + 1) * 128]

        m = {"xT0": xT0, "idxw": idxw, "basis": basis.astype(BF)}
        m.update(shared)
        in_maps.append(m)
    return in_maps


# ------------------------------------------------------------- device build
def build_program(reps=1, mode="full", skip=()):
    import concourse.bass as bass
    import concourse.bacc as bacc
    import concourse.mybir as mybir
    import concourse.tile as tile
    from concourse import library_config

    f32, bf16, i16 = mybir.dt.float32, mybir.dt.bfloat16, mybir.dt.int16
    AF = mybir.ActivationFunctionType
    OP = mybir.AluOpType

    noc = mode.endswith("_noag") or mode.endswith("_noc")
    nc = bacc.Bacc("TRN2", target_bir_lowering=False, debug=False,
                   num_devices=NCORES)

    ins = {
        "xT0": nc.dram_tensor("xT0", [DIM, SHARD], f32, kind="ExternalInput").ap(),
        "idxw": nc.dram_tensor("idxw", [128, NT * 128], i16, kind="ExternalInput").ap(),
        "basis": nc.dram_tensor("basis", [128, 8192], bf16, kind="ExternalInput").ap(),
        "rhs7": nc.dram_tensor("rhs7", [128, DEPTH * 64], bf16, kind="ExternalInput").ap(),
        "ssb": nc.dram_tensor("ssb", [128, NSLOT * 128], bf16, kind="ExternalInput").ap(),
        "wproj": nc.dram_tensor("wproj", [128, DEPTH * 2 * DIM], bf16, kind="ExternalInput").ap(),
        "w1": nc.dram_tensor("w1", [128, 3 * 2 * HID], bf16, kind="ExternalInput").ap(),
        "w2": nc.dram_tensor("w2", [128, 3 * 8 * DIM], bf16, kind="ExternalInput").ap(),
        "b1": nc.dram_tensor("b1", [128, 3 * 8], f32, kind="ExternalInput").ap(),
        "mg": nc.dram_tensor("mg", [128, 3 * 2], f32, kind="ExternalInput").ap(),
        "mb": nc.dram_tensor("mb", [128, 3 * 2], f32, kind="ExternalInput").ap(),
        "lg": nc.dram_tensor("lg", [128, DEPTH * 2], f32, kind="ExternalInput").ap(),
        "lb": nc.dram_tensor("lb", [128, DEPTH * 2], f32, kind="ExternalInput").ap(),
    }
    xout = nc.dram_tensor("xout", [DIM, SHARD], f32, kind="ExternalOutput").ap()

    with tile.TileContext(nc) as tc:
        nc.gpsimd.load_library(library_config.mlp)
        with tc.tile_pool(name="const", bufs=1) as cpool, \
             tc.tile_pool(name="state", bufs=1) as spool, \
             tc.tile_pool(name="stage", bufs=1) as stpool, \
             tc.tile_pool(name="deep", bufs=3) as dppool, \
             tc.tile_pool(name="psum", bufs=1, space="PSUM") as pspool, \
             tc.tile_pool(name="dram", bufs=2, space="DRAM") as dpool, \
             tc.tile_pool(name="sdram", bufs=4, space="DRAM") as sdpool:

            # ---- constants in SBUF
            c_idx = cpool.tile([128, NT * 128], i16, tag="idx")
            c_bas = cpool.tile([128, 8192], bf16, tag="bas")
            c_r7 = cpool.tile([128, DEPTH * 64], bf16, tag="r7")
            c_s = cpool.tile([128, NSLOT * 128], bf16, tag="s")
            c_wp = cpool.tile([128, DEPTH * 2 * DIM], bf16, tag="wp")
            c_w1 = cpool.tile([128, 3 * 2 * HID], bf16, tag="w1")
            c_w2 = cpool.tile([128, 3 * 8 * DIM], bf16, tag="w2")
            c_b1 = cpool.tile([128, 3 * 8], f32, tag="b1")
            c_mg = cpool.tile([128, 3 * 2], f32, tag="mg")
            c_mb = cpool.tile([128, 3 * 2], f32, tag="mb")
            c_lg = cpool.tile([128, DEPTH * 2], f32, tag="lg")
            c_lb = cpool.tile([128, DEPTH * 2], f32, tag="lb")
            for t_, name in ((c_idx, "idxw"), (c_bas, "basis"), (c_r7, "rhs7"),
                             (c_s, "ssb"), (c_wp, "wproj"), (c_w1, "w1"),
                             (c_w2, "w2"), (c_b1, "b1"), (c_mg, "mg"),
                             (c_mb, "mb"), (c_lg, "lg"), (c_lb, "lb")):
                nc.sync.dma_start(t_[:], ins[name][:])

            # ---- state
            xT = spool.tile([128, 2, SHARD], f32, tag="xT")
            xTb = spool.tile([128, 2, SHARD], bf16, tag="xTb")

            def refresh_xtb():
                for ct in range(2):
                    nc.vector.tensor_copy(xTb[:, ct, :], xT[:, ct, :])

            def bn_stats_allreduce(sum_src, sq_src):
                """sum_src/sq_src: [128, 2] f32 APs of per-core partials.
                Returns (s_ap, t_ap) [128, 2] f32 tiles (gamma/..., beta/...)
                factors; caller multiplies/adds."""
                st = spool.tile([128, 4], f32, tag="stpack")
                nc.vector.tensor_copy(st[:, 0:2], sum_src)
                nc.vector.tensor_copy(st[:, 2:4], sq_src)
                stg = spool.tile([128, 4], f32, tag="stglob")
                if noc:
                    # debug: local stats scaled up as a stand-in
                    nc.vector.tensor_scalar_mul(stg[:], st[:], float(NCORES))
                    return stg
                d_in = sdpool.tile([128, 4], f32, tag="st_in")
                d_out = sdpool.tile([128, 4], f32, tag="st_out")
                nc.sync.dma_start(d_in[:], st[:])
                nc.gpsimd.collective_compute(
                    "AllReduce", OP.add,
                    ins=[d_in.opt()], outs=[d_out.opt()],
                    replica_groups=[list(range(NCORES))],
                )
                nc.sync.dma_start(stg[:], d_out[:])
                return stg

            def bn_finalize(stg, gam_ap, bet_ap):
                mu = spool.tile([128, 2], f32, tag="bn_mu")
                var = spool.tile([128, 2], f32, tag="bn_var")
                sfac = spool.tile([128, 2], f32, tag="bn_s")
                tfac = spool.tile([128, 2], f32, tag="bn_t")
                nc.vector.tensor_scalar_mul(mu[:], stg[:, 0:2], 1.0 / (B * N))
                # var = msq - mu^2 ; sd = sqrt(var+EPS); s = gamma/sd; t = beta-s*mu
                nc.vector.tensor_scalar_mul(var[:], stg[:, 2:4], 1.0 / (B * N))
                sq = spool.tile([128, 2], f32, tag="bn_sq")
                nc.vector.tensor_tensor(sq[:], mu[:], mu[:], OP.mult)
                nc.vector.tensor_tensor(var[:], var[:], sq[:], OP.subtract)
                nc.vector.tensor_scalar_add(var[:], var[:], EPS)
                # rsqrt = exp(-0.5*ln(var)) -- Ln/Exp share one ACT table
                # set (natural_log_exp), avoiding a Sqrt-set swap per BN
                lnv = spool.tile([128, 2], f32, tag="bn_ln")
                nc.scalar.activation(lnv[:], var[:], AF.Ln)
                inv = spool.tile([128, 2], f32, tag="bn_inv")
                nc.scalar.activation(inv[:], lnv[:], AF.Exp, scale=-0.5)
                nc.vector.tensor_tensor(sfac[:], gam_ap, inv[:], OP.mult)
                nc.vector.tensor_tensor(tfac[:], sfac[:], mu[:], OP.mult)
                nc.vector.tensor_tensor(tfac[:], bet_ap, tfac[:], OP.subtract)
                return sfac, tfac

            def apply_update(src_view, sfac, tfac):
                """xT += s*src + t ; refresh xTb. src_view(ct) -> AP whose free
                element count is SHARD (any dim structure). Emitted in 512-col
                chunks so downstream per-tile consumers (proj) can pipeline."""
                CH = 512
                for ct in range(2):
                    sv = src_view(ct)
                    for q in range(SHARD // CH):
                        cs = slice(q * CH, (q + 1) * CH)
                        if sv.ndim == 3:
                            svq = sv[:, 4 * q:4 * (q + 1), :]
                            xv = xT[:, ct, cs].rearrange("p (a j) -> p a j", j=128)
                        else:
                            svq = sv[:, cs]
                            xv = xT[:, ct, cs]
                        nc.vector.scalar_tensor_tensor(
                            xv, svq, sfac[:, ct:ct + 1], xv, OP.mult, OP.add)
                        nc.vector.tensor_scalar_add(
                            xT[:, ct, cs], xT[:, ct, cs], tfac[:, ct:ct + 1])
                        nc.vector.tensor_copy(xTb[:, ct, cs], xT[:, ct, cs])

            def mlp(j):
                h1b = stpool.tile([128, 8, CHN], bf16, tag="h1b", bufs=2)
                h2b = stpool.tile([128, 2, SHARD], bf16, tag="h2b")
                junk = stpool.tile([128, CHN], bf16, tag="junk")
                sums = stpool.tile([128, 2, NCH], f32, tag="msum")
                sqs = stpool.tile([128, 2, NCH], f32, tag="msq")
                for nch in range(NCH):
                    n0 = nch * CHN
                    for ht in range(8):
                        p1 = pspool.tile([128, CHN], mybir.dt.float32, tag="pa", bufs=2)
                        for kt in range(2):
                            nc.tensor.matmul(
                                p1[:],
                                c_w1[:, (j * 2 + kt) * HID + ht * 128:
                                     (j * 2 + kt) * HID + (ht + 1) * 128],
                                xTb[:, kt, n0:n0 + CHN],
                                start=(kt == 0), stop=(kt == 1))
                        nc.scalar.activation(h1b[:, ht, :], p1[:],
                                             AF.Gelu_apprx_tanh,
                                             bias=c_b1[:, j * 8 + ht:j * 8 + ht + 1])
                    for ct in range(2):
                        p2 = pspool.tile([128, CHN], mybir.dt.float32, tag="pb", bufs=2)
                        for ht in range(8):
                            nc.tensor.matmul(
                                p2[:],
                                c_w2[:, (j * 8 + ht) * DIM + ct * 128:
                                     (j * 8 + ht) * DIM + (ct + 1) * 128],
                                h1b[:, ht, :],
                                start=(ht == 0), stop=(ht == 7))
                        nc.scalar.activation(
                            h2b[:, ct, n0:n0 + CHN], p2[:], AF.Copy,
                            accum_out=sums[:, ct, nch:nch + 1])
                        nc.vector.scalar_tensor_tensor(
                            junk[:], h2b[:, ct, n0:n0 + CHN], 1.0,
                            h2b[:, ct, n0:n0 + CHN], OP.mult, OP.mult,
                            accum_out=sqs[:, ct, nch:nch + 1])
                rsum = stpool.tile([128, 2], f32, tag="mrsum")
                rsq = stpool.tile([128, 2], f32, tag="mrsq")
                nc.vector.tensor_reduce(rsum[:], sums[:], mybir.AxisListType.X, OP.add)
                nc.vector.tensor_reduce(rsq[:], sqs[:], mybir.AxisListType.X, OP.add)
                stg = bn_stats_allreduce(rsum[:], rsq[:])
                sfac, tfac = bn_finalize(stg, c_mg[:, j * 2:j * 2 + 2],
                                         c_mb[:, j * 2:j * 2 + 2])
                apply_update(lambda ct: h2b[:, ct, :], sfac, tfac)

            def lfp(l):
                hsh = stpool.tile([128, NT, DIM], bf16, tag="hsh")
                # 1) proj h-shard row-major, then AllGather into the table
                for t in range(NT):
                    ph = pspool.tile([128, DIM], mybir.dt.float32, tag="pa", bufs=2)
                    for kt in range(2):
                        nc.tensor.matmul(
                            ph[:],
                            xTb[:, kt, t * 128:(t + 1) * 128],
                            c_wp[:, (l * 2 + kt) * DIM:(l * 2 + kt + 1) * DIM],
                            start=(kt == 0), stop=(kt == 1))
                    nc.scalar.activation(hsh[:, t, :], ph[:], AF.Copy)
                bounce = dpool.tile([SHARD, DIM], bf16, tag="bounce")
                table = dpool.tile([N, DIM], bf16, tag="table")
                nc.sync.dma_start(
                    bounce.rearrange("(t p) c -> p t c", p=128), hsh[:])
                if noc:
                    # stand-in: own shard only (models the local table write)
                    nc.sync.dma_start(table[0:SHARD, :], bounce[:])
                else:
                    nc.gpsimd.collective_compute(
                        "AllGather", OP.bypass,
                        ins=[bounce.opt()], outs=[table.opt()],
                        replica_groups=[[0, 1, 2, 3], [4, 5, 6, 7]],
                    )
                # 2) per-tile: d2 -> exp -> gather -> multiply -> select
                aggsb = stpool.tile([128, NT, DIM], bf16, tag="aggsb")
                for t in range(NT):
                    # 4-way row-group concurrency: each concurrent group must
                    # write a distinct PSUM bank (same-bank concurrent PE
                    # writes fault the exec unit). slot s -> bank s%4, 64-col
                    # sub-offset s//4.
                    pd2 = pspool.tile([128, 4, 512], mybir.dt.float32, tag="pd2", bufs=1)
                    for s in range(NSLOT if "d2" not in skip else 1):
                        sg = t * NSLOT + s
                        rg, cb = sg % 4, sg // 4
                        nc.tensor.matmul(
                            pd2[:, s % 4, (s // 4) * 64:(s // 4 + 1) * 64],
                            c_bas[32 * rg:32 * rg + 7, cb * 128:(cb + 1) * 128],
                            c_r7[32 * rg:32 * rg + 7, l * 64:(l + 1) * 64],
                            start=True, stop=True,
                            tile_position=(32 * rg, 0))
                    if "d2" in skip:
                        pass
                    wgt = dppool.tile([128, NSLOT * 64], bf16, tag="wgt")
                    # wgt col (q*4+s4)*64+g <- pd2[:, s4, q*64+g]
                    if "exp" not in skip:
                        nc.scalar.activation(
                            wgt[:].rearrange("p (q s4 g) -> p s4 q g", s4=4, g=64),
                            pd2[:, :, 0:256].rearrange("p s4 (q g) -> p s4 q g", g=64),
                            AF.Exp)
                    hn = dppool.tile([128, NSLOT, DIM], bf16, tag="hn")
                    if "gather" not in skip:
                      nc.gpsimd.dma_gather(
                        out_ap=hn[:],
                        in_ap=table[:],
                        idxs_ap=c_idx[:, t * 128:(t + 1) * 128],
                        num_idxs=ROWS_T,
                        num_idxs_reg=ROWS_T,
                        elem_size=DIM,
                        single_packet=False,
                      )
                    tmp = dppool.tile([128, NSLOT, 4, 64], bf16, tag="tmp")
                    wgt_b = (wgt[:].rearrange("p (s g) -> p s g", g=64)
                             .unsqueeze(2).broadcast_to([128, NSLOT, 4, 64]))
                    if "mult" not in skip:
                        nc.vector.tensor_tensor(
                            tmp[:], hn[:].rearrange("p s (c4 g) -> p s c4 g", g=64),
                            wgt_b, OP.mult)
                    pag = pspool.tile([128, DIM], mybir.dt.float32, tag="pb", bufs=2)
                    for s in range((NSLOT) if "select" not in skip else 1):
                        nc.tensor.matmul(
                            pag[:],
                            c_s[:, s * 128:(s + 1) * 128],
                            tmp[:, s, :, :],
                            start=(s == 0),
                            stop=(s == (NSLOT - 1 if "select" not in skip else 0)))
                    nc.scalar.activation(aggsb[:, t, :], pag[:], AF.Copy)
                # 3) transpose to channel-major (tile-interleaved: [p, tr, ct, j]),
                #    stats, BN, residual
                aggT = stpool.tile([128, NT, 2, 128], bf16, tag="aggT")
                for g4 in range(NT // 4):
                    nc.sync.dma_start_transpose(
                        aggT[:, g4 * 4:(g4 + 1) * 4, :, :]
                        .rearrange("p tr ct j -> p (tr ct) j"),
                        aggsb[:, g4 * 4:(g4 + 1) * 4, :])
                lsum = stpool.tile([128, 2], f32, tag="lsum")
                lsq = stpool.tile([128, 2], f32, tag="lsq")
                ljunk = stpool.tile([128, NT, 128], bf16, tag="ljunk")
                for ct in range(2):
                    nc.vector.tensor_reduce(lsum[:, ct:ct + 1], aggT[:, :, ct, :],
                                            mybir.AxisListType.XY, OP.add)
                    nc.vector.scalar_tensor_tensor(
                        ljunk[:], aggT[:, :, ct, :], 1.0, aggT[:, :, ct, :],
                        OP.mult, OP.mult, accum_out=lsq[:, ct:ct + 1])
                stg = bn_stats_allreduce(lsum[:], lsq[:])
                sfac, tfac = bn_finalize(stg, c_lg[:, l * 2:l * 2 + 2],
                                         c_lb[:, l * 2:l * 2 + 2])
                apply_update(lambda ct: aggT[:, :, ct, :], sfac, tfac)

            do_mlp = mode in ("full", "mlp0", "full_noc")
            do_lfp = mode in ("full", "lfp0", "lfp0_noag", "full_noc")
            n_lfp = DEPTH if mode in ("full", "full_noc") else (1 if do_lfp else 0)
            for rep in range(reps):
                for ct in range(2):
                    nc.sync.dma_start(xT[:, ct, :],
                                      ins["xT0"][ct * 128:(ct + 1) * 128, :])
                refresh_xtb()
                if do_mlp:
                    mlp(0)
                for l in range(n_lfp):
                    lfp(l)
                    if l % 2 == 1 and mode in ("full", "full_noc"):
                        mlp(1 + l // 2)
            nc.sync.dma_start(xout.rearrange("(c p) n -> p c n", p=128), xT[:])

    nc.compile()
    return nc


_NC_CACHE = {}


def _get_nc(reps=1, mode="full"):
    key = (reps, mode)
    if key not in _NC_CACHE:
        _NC_CACHE[key] = build_program(reps, mode)
    return _NC_CACHE[key]


def run_on_cores(in_maps, reps=1, mode="full"):
    from concourse.bass_utils import run_bass_kernel_spmd
    nc = _get_nc(reps, mode)
    return run_bass_kernel_spmd(nc, in_maps, core_ids=list(range(NCORES)))


def kernel(**inputs):
    in_maps = _pack_inputs(inputs)
    res = None
    for attempt in range(4):
        try:
            res = run_on_cores(in_maps, reps=1)
            break
        except Exception:
            # transient device-state faults occur on this fleet; back off and
            # retry on a fresh dispatch (observed to clear them)
            if attempt == 3:
                raise
            import time as _time
            _time.sleep(5.0)
            try:
                import jax
                jax.clear_caches()
            except Exception:
                pass
    out = np.zeros((B, N, DIM), np.float32)
    for core in range(NCORES):
        b, sh = core // 4, core % 4
        out[b, sh * SHARD:(sh + 1) * SHARD] = res.results[core]["xout"].T[:, PERM_INV]
    return out.astype(np.float32)

